# revision 1
# baseline (speedup 1.0000x reference)
"""PixelAttention Trainium2 kernel.

Computes, for each batch image (data-parallel, one image per NeuronCore):
    seq  = image.reshape(C, T).T            # [T, C], T = 32*32
    kqv  = seq @ w_kqv + b_kqv
    per-head causal attention (8 heads, head_dim 32), softmax over keys
    out  = mix(attn) + b_mix + image

Layout strategy (per core):
  - x kept as [C, T] (channels on partitions) -- image memory order directly.
  - k^T, q^T computed as [j, t]; QK^T contractions (K=32) run 4 heads
    row-packed in the PE array; logits land transposed L[s, t] so softmax
    normalization is a matmul-with-ones fold rather than a free-dim reduce.
  - exp on ScalarE (psum -> sbuf bf16); causal masking of diagonal blocks is
    a DVE/GpSimd multiply by an upper-triangular 0/1 tile (keeps PE streams
    geometry-uniform -- interleaving K=1/K=128 matmuls into the K=32 stream
    destroys PE pipelining and keeps the HAM clock gate cold).
  - out_un^T = [V | 1]^T E accumulated over s-tiles (M=33: psum row 32 is the
    softmax denominator), 2 heads col-packed per matmul.
  - division by denominator via DVE reciprocal + selector-matmul broadcast;
    the division tail and mix run after both chunks so PSUM slot rotation
    never blocks the attention stream.
  - mix as w_mix^T @ attn^T; biases and residual folded into DVE ops.

All matmul operands are bf16 (fp32 PSUM accumulation); residual path fp32.
"""

import numpy as np
import ml_dtypes

import concourse.bass as bass
import concourse.tile as tile
from concourse import bacc, mybir
from concourse.bass_utils import run_bass_kernel_spmd

BF = ml_dtypes.bfloat16
T, C, H, D = 1024, 256, 8, 32
N_CORES = 8

_CACHE = {}


def _build_nc():
    f32 = mybir.dt.float32
    bf16 = mybir.dt.bfloat16
    EXP = mybir.ActivationFunctionType.Exp
    ADD = mybir.AluOpType.add

    nc = bacc.Bacc("TRN2", target_bir_lowering=False, debug=False)

    def din(name, shape, dt):
        return nc.dram_tensor(name, shape, dt, kind="ExternalInput").ap()

    x_bf = din("x_bf", [C, T], bf16)
    x_f = din("x_f", [C, T], f32)
    wk = din("wk", [C, 256], bf16)
    wq = din("wq", [C, 256], bf16)
    wv = din("wv", [C, 256], bf16)
    wm = din("wm", [C, 256], bf16)
    bjt = din("bjt", [4, 128], f32)   # per-partition bias for qkT tiles
    bv = din("bv", [1, 256], bf16)    # broadcast along partitions
    bm2 = din("bm2", [2, 128], f32)   # per-partition bias for mix output
    tri = din("tri", [128, 128], bf16)  # tri[p, q] = 1 if q >= p else 0
    sel = din("sel", [8, 256], bf16)
    y = nc.dram_tensor("y", [C, T], f32, kind="ExternalOutput").ap()

    with tile.TileContext(nc) as tc:
        with (
            tc.tile_pool(name="consts", bufs=1) as consts,
            tc.tile_pool(name="sb", bufs=6) as sb,
            tc.tile_pool(name="ps", bufs=2, space="PSUM") as ps_pool,
            tc.tile_pool(name="lpp", bufs=3, space="PSUM") as lp_pool,
            tc.tile_pool(name="dr", bufs=2, space="DRAM") as dram_pool,
        ):
            # ---- constant loads (critical path first; bulk on other queues) ----
            xb = consts.tile([128, 2, T], bf16)
            x_bf_r = x_bf.rearrange("(a p) t -> p a t", p=128)
            nc.sync.dma_start(out=xb[:, 0, :], in_=x_bf_r[:, 0, :])
            nc.sync.dma_start(out=xb[:, 1, :], in_=x_bf_r[:, 1, :])
            w_tiles = {}
            for eng, (name, ap) in zip(
                (nc.sync, nc.sync, nc.scalar, nc.scalar),
                (("wk", wk), ("wq", wq), ("wv", wv), ("wm", wm)),
            ):
                t_ = consts.tile([128, 2, 256], bf16, tag=name, name=name)
                eng.dma_start(out=t_, in_=ap.rearrange("(a p) j -> p a j", p=128))
                w_tiles[name] = t_
            bjt_sb = consts.tile([128, 4], f32)
            nc.scalar.dma_start(out=bjt_sb, in_=bjt.rearrange("a p -> p a"))
            bv_bc = consts.tile([128, 256], bf16)
            nc.scalar.dma_start(
                out=bv_bc,
                in_=bass.AP(tensor=bv.tensor, offset=bv.offset,
                            ap=[[0, 128]] + list(bv.ap[1:])),
            )
            tri_sb = consts.tile([128, 128], bf16)
            nc.gpsimd.dma_start(out=tri_sb, in_=tri)
            sel_s = consts.tile([8, 256], bf16)
            nc.gpsimd.dma_start(out=sel_s, in_=sel)
            xf = consts.tile([128, 2, T], f32)
            nc.gpsimd.dma_start(out=xf, in_=x_f.rearrange("(a p) t -> p a t", p=128))
            bm2_sb = consts.tile([128, 2], f32)
            nc.gpsimd.dma_start(out=bm2_sb, in_=bm2.rearrange("a p -> p a"))

            warm = consts.tile([128, 1], f32, tag="warm", name="warm")
            nc.scalar.activation(out=warm, in_=bjt_sb[:, 0:1], func=EXP)

            qkT = consts.tile([128, 4, T], bf16)  # jt 0-1: kT, 2-3: qT
            vsb = consts.tile([128, 8, H, 33], bf16)  # [p, s_tile, head, v|1]
            nc.vector.memset(vsb[:, :, :, 32:33], 1.0)
            attnT = consts.tile([128, 2, T], bf16)

            # ---- projections: g=0 eager; g=1 and V woven into attention ----
            def proj_group(dst, wname, jl, tch):
                w_s = w_tiles[wname]
                js = slice(jl * 128, (jl + 1) * 128)
                ts_ = slice(tch * 512, (tch + 1) * 512)
                p = lp_pool.tile([128, 2, 512], f32, tag="lp",
                                 name="pp")[:, 0, :]
                nc.tensor.matmul(
                    out=p, lhsT=w_s[:, 0, js], rhs=xb[:, 0, ts_],
                    start=True, stop=False,
                )
                nc.tensor.matmul(
                    out=p, lhsT=w_s[:, 1, js], rhs=xb[:, 1, ts_],
                    start=False, stop=True,
                )
                nc.vector.tensor_scalar(
                    out=qkT[:, dst, ts_], in0=p,
                    scalar1=bjt_sb[:, dst:dst + 1], scalar2=None, op0=ADD,
                )

            def v_group(st):
                ss = slice(st * 128, (st + 1) * 128)
                p = lp_pool.tile([128, 2, 512], f32, tag="lp",
                                 name="pv")[:, 0, 0:256]
                nc.tensor.matmul(
                    out=p, lhsT=xb[:, 0, ss], rhs=w_tiles["wv"][:, 0, :],
                    start=True, stop=False,
                )
                nc.tensor.matmul(
                    out=p, lhsT=xb[:, 1, ss], rhs=w_tiles["wv"][:, 1, :],
                    start=False, stop=True,
                )
                nc.vector.tensor_add(
                    out=vsb[:, st, :, 0:32],
                    in0=p.rearrange("p (h e) -> p h e", e=32),
                    in1=bv_bc.rearrange("p (h e) -> p h e", e=32),
                )

            for tch in range(2):
                proj_group(0, "wk", 0, tch)
            for tch in range(2):
                proj_group(2, "wq", 0, tch)
            g1_projs = [(1, "wk", 1, 0), (1, "wk", 1, 1),
                        (3, "wq", 1, 0), (3, "wq", 1, 1)]

            # ---- phase 3: attention ----
            pending = []
            for c in range(2):
                ouc = {}
                rs = sb.tile([8, 512], f32, tag="rs", name="rs", bufs=2)
                for g in range(2):
                    po = {
                        0: ps_pool.tile([128, 512], f32, tag="po", name="po0"),
                        1: ps_pool.tile([128, 512], f32, tag="po", name="po1"),
                    }
                    for st in range(4 * c + 4):
                        if c == 0 and g == 0:
                            v_group(st)
                            if g1_projs:
                                proj_group(*g1_projs.pop(0))
                        elif c == 0 and g == 1:
                            v_group(4 + st)
                        tlo = 128 * st - 512 * c
                        diag = tlo >= 0
                        if not diag:
                            tlo = 0
                        for pair in range(2):
                            lp = lp_pool.tile([128, 2, 512], f32, tag="lp", name="lp")
                            E = sb.tile([128, 2, 512], bf16, tag="E", name="E", bufs=8)
                            for h2 in range(2):
                                hl = 2 * pair + h2
                                rp = 32 * hl
                                kT_l = qkT[rp:rp + 32, g, st * 128:(st + 1) * 128]
                                qg = qkT[rp:rp + 32, 2 + g, :]
                                nc.tensor.matmul(
                                    out=lp[:, h2, tlo:512], lhsT=kT_l,
                                    rhs=qg[:, c * 512 + tlo:(c + 1) * 512],
                                    start=True, stop=True,
                                    tile_position=(rp, 0),
                                )
                            nc.scalar.activation(
                                out=E[:, :, tlo:512], in_=lp[:, :, tlo:512],
                                func=EXP,
                            )
                            if diag:
                                tri_b = bass.AP(
                                    tensor=tri_sb.tensor, offset=tri_sb.offset,
                                    ap=[list(tri_sb.ap[0]), [0, 2]]
                                       + list(tri_sb.ap[1:]),
                                )
                                nc.gpsimd.tensor_mul(
                                    out=E[:, :, tlo:tlo + 128],
                                    in0=E[:, :, tlo:tlo + 128],
                                    in1=tri_b,
                                )
                            for h2 in range(2):
                                hl = 2 * pair + h2
                                h = 4 * g + hl
                                nc.tensor.matmul(
                                    out=po[pair][64 * h2:64 * h2 + 33, tlo:512],
                                    lhsT=vsb[:, st, h, :], rhs=E[:, h2, tlo:512],
                                    start=(st == 0), stop=(st == 4 * c + 3),
                                    skip_group_check=True,
                                    tile_position=(0, 64 * h2),
                                )
                    # division part 1 for this (c, g): psum -> sbuf, compact
                    t_ = sb.tile([128, 512], f32, tag="ouc", name="ouc", bufs=4)
                    ou = {}
                    for pair in range(2):
                        t_ou = sb.tile([128, 512], f32, tag="ou", name="ou")
                        nc.vector.tensor_copy(out=t_ou, in_=po[pair])
                        ou[pair] = t_ou
                    for pair in range(2):
                        for h2 in range(2):
                            hl = 2 * pair + h2
                            h = 4 * g + hl
                            nc.gpsimd.dma_start(
                                out=t_[32 * hl:32 * hl + 32, :],
                                in_=ou[pair][64 * h2:64 * h2 + 32, :],
                            )
                            nc.sync.dma_start(
                                out=rs[h:h + 1, :],
                                in_=ou[pair][64 * h2 + 32:64 * h2 + 33, :],
                            )
                    ouc[g] = t_
                rcf = sb.tile([8, 512], f32, tag="rcf", name="rcf", bufs=2)
                nc.vector.reciprocal_approx_fast(out=rcf, in_=rs)
                rcb = sb.tile([8, 512], bf16, tag="rcb", name="rcb", bufs=2)
                nc.vector.tensor_copy(out=rcb, in_=rcf)
                pending.append((c, ouc, rcb))

            # ---- division part 2 + mix + residual (off the PE critical path) ----
            for c, ouc, rcb in pending:
                cs = slice(c * 512, (c + 1) * 512)
                for g in range(2):
                    bc = ps_pool.tile([128, 512], f32, tag="po", name="bc")
                    nc.tensor.matmul(
                        out=bc, lhsT=sel_s[:, g * 128:(g + 1) * 128], rhs=rcb,
                        start=True, stop=True,
                    )
                    nc.vector.tensor_mul(out=attnT[:, g, cs], in0=ouc[g], in1=bc)
                for c2t in range(2):
                    c2s = slice(c2t * 128, (c2t + 1) * 128)
                    mp = lp_pool.tile([128, 2, 512], f32, tag="lp",
                                      name="mp")[:, 0, :]
                    nc.tensor.matmul(
                        out=mp, lhsT=w_tiles["wm"][:, 0, c2s], rhs=attnT[:, 0, cs],
                        start=True, stop=False,
                    )
                    nc.tensor.matmul(
                        out=mp, lhsT=w_tiles["wm"][:, 1, c2s], rhs=attnT[:, 1, cs],
                        start=False, stop=True,
                    )
                    os_ = sb.tile([128, 512], f32, tag="os", name="os")
                    nc.vector.scalar_tensor_tensor(
                        out=os_, in0=mp, scalar=bm2_sb[:, c2t:c2t + 1],
                        in1=xf[:, c2t, cs], op0=ADD, op1=ADD,
                    )
                    nc.sync.dma_start(
                        out=y.rearrange("(a p) t -> p a t", p=128)[:, c2t, cs],
                        in_=os_,
                    )

    nc.compile()
    return nc


def _host_inputs(image, w_kqv, b_kqv, w_mix, b_mix):
    s = np.float32(1.0 / np.sqrt(D))
    wk = w_kqv[:, :256]
    wq = w_kqv[:, 256:512] * s
    wv = w_kqv[:, 512:]
    bk = b_kqv[:256].astype(np.float32)
    bq = (b_kqv[256:512] * s).astype(np.float32)
    bv = b_kqv[512:].reshape(1, 256)
    bjt = np.stack([bk[0:128], bk[128:256], bq[0:128], bq[128:256]])
    bm2 = np.asarray(b_mix, np.float32).reshape(2, 128)
    tri = (np.arange(128)[None, :] >= np.arange(128)[:, None]).astype(np.float32)
    sel = np.zeros((8, 256), np.float32)
    for h in range(8):
        g, hl = divmod(h, 4)
        sel[h, 128 * g + 32 * hl:128 * g + 32 * hl + 32] = 1.0
    common = {
        "wk": np.ascontiguousarray(wk).astype(BF),
        "wq": np.ascontiguousarray(wq).astype(BF),
        "wv": np.ascontiguousarray(wv).astype(BF),
        "wm": np.ascontiguousarray(w_mix).astype(BF),
        "bjt": np.ascontiguousarray(bjt),
        "bv": bv.astype(BF),
        "bm2": bm2,
        "tri": tri.astype(BF),
        "sel": sel.astype(BF),
    }
    in_maps = []
    for i in range(N_CORES):
        x = np.ascontiguousarray(image[i].reshape(C, T)).astype(np.float32)
        in_maps.append({**common, "x_f": x, "x_bf": x.astype(BF)})
    return in_maps


def _run(inputs, trace=False):
    if "nc" not in _CACHE:
        _CACHE["nc"] = _build_nc()
    nc = _CACHE["nc"]
    in_maps = _host_inputs(
        np.asarray(inputs["image"], np.float32),
        np.asarray(inputs["w_kqv"], np.float32),
        np.asarray(inputs["b_kqv"], np.float32),
        np.asarray(inputs["w_mix"], np.float32),
        np.asarray(inputs["b_mix"], np.float32),
    )
    res = run_bass_kernel_spmd(nc, in_maps, list(range(N_CORES)), trace=trace)
    out = np.stack(
        [np.asarray(res.results[i]["y"]).reshape(C, 32, 32) for i in range(N_CORES)]
    ).astype(np.float32)
    return out, res


def kernel(**inputs):
    out, _ = _run(inputs, trace=False)
    return out



# revision 15
# speedup vs baseline: 1.0549x; 1.0549x over previous
"""PixelAttention Trainium2 kernel (v2).

Data-parallel: one image per NeuronCore. Per core:
    seq  = image.reshape(C, T).T            # [T, C], T = 32*32
    kqv  = seq @ w_kqv + b_kqv
    per-head causal attention (8 heads, head_dim 32), softmax over keys
    out  = mix(attn) + b_mix + image

Key design points (vs v1 baseline at ~87us):
  - exp is split across ScalarE (true Exp) and DVE (one-instruction
    Schraudolph fast-exp: int16(L*2^7/ln2 + (127*128-5.5)) bitcast to bf16,
    max rel err ~3.3%, end-to-end ~2e-3).
  - causal diag masking: ScalarE-assigned tiles get -60 added to the masked
    triangle via an eye-matmul PSUM accumulate (PE); DVE-assigned tiles fold
    the mask into the fast-exp via scalar_tensor_tensor with a const tile.
  - AV matmuls use M=64 weights [V | ones | zeros*31] so all 128 PSUM rows
    are written (denominator at rows 32/96, junk rows exactly zero); the
    division then runs directly on the PO layout, no compaction DMAs.
  - kqv biases ride on the psum->sbuf mover ops; v-bias is folded into the
    mix bias host-side (bm_eff = b_mix + bv @ w_mix).
  - software pipeline: logits(st) issue before AV(st-1) so the PE never
    stalls on exp and stays at full pstate.
"""

import numpy as np
import ml_dtypes

import concourse.bass as bass
import concourse.tile as tile
from concourse import bacc, mybir
from concourse.bass_utils import run_bass_kernel_spmd

BF = ml_dtypes.bfloat16
T, C, H, D = 1024, 256, 8, 32
N_CORES = 8

EXP_A = float(2.0**7 / np.log(2.0))        # 184.66496
EXP_B = float(127 * 128 - 5.5)             # 16250.5
MASK60 = 60.0                              # causal mask additive offset

_CACHE = {}


def _unit_engine(c, g, st, pair):
    """Which engine computes exp for unit (c, g, st, pair).

    'A' = ScalarE true exp (diag tiles masked by Pool tri-multiply on E),
    'D' = DVE fast-exp (diag mask folded into scalar_tensor_tensor).
    """
    return "A" if pair == 0 else "D"


def _build_nc():
    f32 = mybir.dt.float32
    bf16 = mybir.dt.bfloat16
    i16 = mybir.dt.int16
    EXP = mybir.ActivationFunctionType.Exp
    IDENT = mybir.ActivationFunctionType.Identity
    ADD = mybir.AluOpType.add
    MULT = mybir.AluOpType.mult

    nc = bacc.Bacc("TRN2", target_bir_lowering=False, debug=False)

    def din(name, shape, dt):
        return nc.dram_tensor(name, shape, dt, kind="ExternalInput").ap()

    x_bf = din("x_bf", [C, T], bf16)
    x_f = din("x_f", [C, T], f32)
    wk = din("wk", [C, 256], bf16)
    wq = din("wq", [C, 256], bf16)     # pre-scaled by 1/sqrt(D)
    wv = din("wv", [C, 256], bf16)
    wmp = din("wmp", [128, 4, 256], bf16)  # po-row layout mix weights
    bjt = din("bjt", [4, 128], f32)    # bk0, bk1, bq0, bq1 (q pre-scaled)
    bm2 = din("bm2", [2, 128], f32)    # b_mix + bv @ w_mix
    tri = din("tri", [128, 128], bf16)   # 1 if t >= s else 0 (keep mask)
    mbf = din("mbf", [128, 512], f32)    # col<128: B - 60*A if s>t else B; else B
    selp = din("selp", [8, 4, 128], bf16)
    y = nc.dram_tensor("y", [C, T], f32, kind="ExternalOutput").ap()

    with tile.TileContext(nc) as tc:
        with (
            tc.tile_pool(name="consts", bufs=1) as consts,
            tc.tile_pool(name="sb", bufs=4) as sb,
            tc.tile_pool(name="lpp", bufs=3, space="PSUM") as lp_pool,
            tc.tile_pool(name="pop", bufs=2, space="PSUM") as po_pool,
        ):
            # ---------------- input DMAs (critical-path order) -------------
            xb = consts.tile([128, 2, T], bf16)
            x_bf_r = x_bf.rearrange("(a p) t -> p a t", p=128)
            # t0-half first on sync+scalar (gates first projection)
            nc.sync.dma_start(out=xb[:, 0, 0:512], in_=x_bf_r[:, 0, 0:512])
            nc.scalar.dma_start(out=xb[:, 1, 0:512], in_=x_bf_r[:, 1, 0:512])
            w_tiles = {}
            for name, ap in (("wk", wk), ("wq", wq), ("wv", wv)):
                t_ = consts.tile([128, 2, 256], bf16, tag=name, name=name)
                nc.gpsimd.dma_start(
                    out=t_, in_=ap.rearrange("(a p) j -> p a j", p=128))
                w_tiles[name] = t_
            bjt_sb = consts.tile([128, 4], f32)
            nc.scalar.dma_start(out=bjt_sb, in_=bjt.rearrange("a p -> p a"))
            tri_sb = consts.tile([128, 128], bf16)
            nc.sync.dma_start(out=tri_sb, in_=tri)
            mbf_sb = consts.tile([128, 512], f32)
            nc.sync.dma_start(out=mbf_sb, in_=mbf)
            nc.sync.dma_start(out=xb[:, 0, 512:T], in_=x_bf_r[:, 0, 512:T])
            nc.scalar.dma_start(out=xb[:, 1, 512:T], in_=x_bf_r[:, 1, 512:T])
            selp_sb = consts.tile([8, 4, 128], bf16)
            nc.sync.dma_start(out=selp_sb, in_=selp)
            bm2_sb = consts.tile([128, 2], f32)
            nc.scalar.dma_start(out=bm2_sb, in_=bm2.rearrange("a p -> p a"))
            wmp_sb = consts.tile([128, 4, 256], bf16)
            nc.gpsimd.dma_start(out=wmp_sb, in_=wmp)
            xf = consts.tile([128, 2, T], f32)
            nc.gpsimd.dma_start(out=xf, in_=x_f.rearrange("(a p) t -> p a t", p=128))

            # warm the exp table before attention needs it
            warm = consts.tile([128, 1], f32, tag="warm", name="warm")
            nc.scalar.activation(out=warm, in_=bjt_sb[:, 0:1], func=EXP)

            qkT = consts.tile([128, 4, T], bf16)  # 0-1: kT g0/g1, 2-3: qT
            vsb = consts.tile([128, 8, H, 64], bf16)  # [p, st, head, V|1|0*31]
            nc.vector.memset(vsb[:, :, :, 32:33], 1.0)
            nc.vector.memset(vsb[:, :, :, 33:64], 0.0)
            attnT = consts.tile([128, 4, T], bf16)    # [po-row, 2g+pair, t]

            # ---------------- projections --------------------------------
            def proj_group(dst, wname, jl, tch, mover):
                w_s = w_tiles[wname]
                js = slice(jl * 128, (jl + 1) * 128)
                ts_ = slice(tch * 512, (tch + 1) * 512)
                p = lp_pool.tile([128, 2, 512], f32, tag="lp", name="pp")[:, 0, :]
                nc.tensor.matmul(out=p, lhsT=w_s[:, 0, js], rhs=xb[:, 0, ts_],
                                 start=True, stop=False)
                nc.tensor.matmul(out=p, lhsT=w_s[:, 1, js], rhs=xb[:, 1, ts_],
                                 start=False, stop=True)
                if mover == "A":
                    nc.scalar.activation(out=qkT[:, dst, ts_], in_=p,
                                         func=IDENT,
                                         bias=bjt_sb[:, dst:dst + 1])
                else:
                    eng = nc.vector if mover == "D" else nc.gpsimd
                    eng.tensor_scalar(out=qkT[:, dst, ts_], in0=p,
                                      scalar1=bjt_sb[:, dst:dst + 1],
                                      scalar2=None, op0=ADD)

            def v_group(st, mover):
                ss = slice(st * 128, (st + 1) * 128)
                p = lp_pool.tile([128, 2, 512], f32, tag="lp",
                                 name="pv")[:, 0, 0:256]
                nc.tensor.matmul(out=p, lhsT=xb[:, 0, ss],
                                 rhs=w_tiles["wv"][:, 0, :],
                                 start=True, stop=False)
                nc.tensor.matmul(out=p, lhsT=xb[:, 1, ss],
                                 rhs=w_tiles["wv"][:, 1, :],
                                 start=False, stop=True)
                src = p.rearrange("p (h e) -> p h e", e=32)
                dstv = vsb[:, st, :, 0:32]
                if mover == "A":
                    nc.scalar.copy(out=dstv, in_=src)
                elif mover == "D":
                    nc.vector.tensor_copy(out=dstv, in_=src)
                else:
                    nc.gpsimd.tensor_copy(out=dstv, in_=src)

            # head phase: only what c0/g0 needs
            proj_group(0, "wk", 0, 0, "A")   # kT g0, t 0:512
            proj_group(2, "wq", 0, 0, "D")   # qT g0, t 0:512
            for st in range(4):
                v_group(st, "A" if st % 2 == 0 else "D")

            # woven work lists: (c, g, st) -> list of thunks
            weave = {}

            def add_weave(c, g, st, fn):
                weave.setdefault((c, g, st), []).append(fn)

            add_weave(0, 0, 0, lambda: proj_group(1, "wk", 1, 0, "A"))
            add_weave(0, 0, 1, lambda: proj_group(3, "wq", 1, 0, "D"))
            add_weave(0, 0, 2, lambda: proj_group(0, "wk", 0, 1, "A"))
            add_weave(0, 0, 3, lambda: proj_group(2, "wq", 0, 1, "D"))
            add_weave(0, 1, 0, lambda: proj_group(1, "wk", 1, 1, "A"))
            add_weave(0, 1, 1, lambda: proj_group(3, "wq", 1, 1, "D"))
            for st in range(4, 8):
                m = "A" if st % 2 == 0 else "D"
                add_weave(0, 1, st - 4, lambda st=st, m=m: v_group(st, m))

            # ---------------- attention ----------------------------------
            ouPO = {}   # (c, g) -> [128, 2, 512] f32 sbuf
            rs = {}     # c -> [8, 512] f32
            for c_ in range(2):
                rs[c_] = sb.tile([8, 512], f32, tag="rs", name=f"rs{c_}", bufs=2)

            def drain_po(c, g, po):
                t_ = sb.tile([128, 2, 512], f32, tag="oup",
                             name=f"ou{c}{g}", bufs=4)
                nc.vector.tensor_copy(out=t_[:, 0, :], in_=po[0])
                nc.scalar.copy(out=t_[:, 1, :], in_=po[1])
                ouPO[(c, g)] = t_
                # denominators: rows 32/96 of each pair -> rs[c] rows
                # rs row r = 4g + 2*h2 + pair  (head = 4g + 2*pair + h2)
                for h2 in range(2):
                    nc.sync.dma_start(
                        out=rs[c][4 * g + 2 * h2: 4 * g + 2 * h2 + 2, :],
                        in_=t_[32 + 64 * h2: 33 + 64 * h2, :, :],
                    )

            rcb = {}

            def recip_c(c):
                rcf = sb.tile([8, 512], f32, tag="rcf", name="rcf", bufs=2)
                nc.vector.reciprocal_approx_fast(out=rcf, in_=rs[c])
                rcb_t = sb.tile([8, 512], bf16, tag="rcb", name="rcb", bufs=2)
                nc.vector.tensor_copy(out=rcb_t, in_=rcf)
                rcb[c] = rcb_t

            po_prev = None  # (c, g, po) awaiting drain
            for c in range(2):
                n_st = 4 + 4 * c
                for g in range(2):
                    po = {
                        0: po_pool.tile([128, 512], f32, tag="po", name="po0"),
                        1: po_pool.tile([128, 512], f32, tag="po", name="po1"),
                    }
                    e_tiles = {}
                    for st in range(n_st + 1):
                        if st < n_st:
                            tlo_r = 128 * st - 512 * c
                            diag = tlo_r >= 0
                            tlo = max(tlo_r, 0)
                            for pair in range(2):
                                eng = _unit_engine(c, g, st, pair)
                                lp = lp_pool.tile([128, 2, 512], f32,
                                                  tag="lp", name="lp")
                                for h2 in range(2):
                                    hl = 2 * pair + h2
                                    rp = 32 * hl
                                    kT_l = qkT[rp:rp + 32, g,
                                               st * 128:(st + 1) * 128]
                                    qg = qkT[rp:rp + 32, 2 + g, :]
                                    nc.tensor.matmul(
                                        out=lp[:, h2, tlo:512], lhsT=kT_l,
                                        rhs=qg[:, c * 512 + tlo:(c + 1) * 512],
                                        start=True, stop=True,
                                        tile_position=(rp, 0),
                                    )
                                E = sb.tile([128, 2, 512], bf16, tag="E",
                                            name="E", bufs=6)
                                if eng == "A":
                                    nc.scalar.activation(
                                        out=E[:, :, tlo:512],
                                        in_=lp[:, :, tlo:512], func=EXP)
                                    if diag:
                                        tri_b = bass.AP(
                                            tensor=tri_sb.tensor,
                                            offset=tri_sb.offset,
                                            ap=[list(tri_sb.ap[0]), [0, 2]]
                                               + list(tri_sb.ap[1:]),
                                        )
                                        nc.gpsimd.tensor_mul(
                                            out=E[:, :, tlo:tlo + 128],
                                            in0=E[:, :, tlo:tlo + 128],
                                            in1=tri_b,
                                        )
                                else:
                                    E16 = E.bitcast(i16)
                                    if diag:
                                        w_ = 512 - tlo
                                        mb_b = bass.AP(
                                            tensor=mbf_sb.tensor,
                                            offset=mbf_sb.offset,
                                            ap=[list(mbf_sb.ap[0]), [0, 2],
                                                [1, w_]],
                                        )
                                        nc.vector.scalar_tensor_tensor(
                                            out=E16[:, :, tlo:512],
                                            in0=lp[:, :, tlo:512],
                                            scalar=EXP_A, in1=mb_b,
                                            op0=MULT, op1=ADD)
                                    else:
                                        nc.vector.tensor_scalar(
                                            out=E16[:, :, tlo:512],
                                            in0=lp[:, :, tlo:512],
                                            scalar1=EXP_A, scalar2=EXP_B,
                                            op0=MULT, op1=ADD)
                                e_tiles[(st, pair)] = E
                            for fn in weave.get((c, g, st), []):
                                fn()
                            if st == 0 and po_prev is not None:
                                drain_po(*po_prev)
                                po_prev = None
                            if c == 1 and g == 0 and st == 5:
                                recip_c(0)
                        if st > 0:
                            stp = st - 1
                            tlo_p = max(128 * stp - 512 * c, 0)
                            for pair in range(2):
                                E = e_tiles.pop((stp, pair))
                                for h2 in range(2):
                                    hl = 2 * pair + h2
                                    h = 4 * g + hl
                                    nc.tensor.matmul(
                                        out=po[pair][64 * h2:64 * h2 + 64,
                                                     tlo_p:512],
                                        lhsT=vsb[:, stp, h, :],
                                        rhs=E[:, h2, tlo_p:512],
                                        start=(stp == 0),
                                        stop=(stp == n_st - 1),
                                        skip_group_check=True,
                                        tile_position=(0, 64 * h2),
                                    )
                    po_prev = (c, g, po)

            # final drain + reciprocal for c1
            drain_po(*po_prev)
            recip_c(1)

            # ---------------- division + mix + residual ------------------
            for c in range(2):
                cs = slice(c * 512, (c + 1) * 512)
                for g in range(2):
                    for pair in range(2):
                        gp = 2 * g + pair
                        bc = lp_pool.tile([128, 2, 512], f32, tag="lp",
                                          name="bc")[:, 0, :]
                        nc.tensor.matmul(out=bc, lhsT=selp_sb[:, gp, :],
                                         rhs=rcb[c], start=True, stop=True)
                        nc.vector.tensor_mul(
                            out=attnT[:, gp, cs],
                            in0=ouPO[(c, g)][:, pair, :], in1=bc)
                for c2t in range(2):
                    c2s = slice(c2t * 128, (c2t + 1) * 128)
                    mp = lp_pool.tile([128, 2, 512], f32, tag="lp",
                                      name="mp")[:, 0, :]
                    for gp in range(4):
                        nc.tensor.matmul(out=mp, lhsT=wmp_sb[:, gp, c2s],
                                         rhs=attnT[:, gp, cs],
                                         start=(gp == 0), stop=(gp == 3))
                    os_ = sb.tile([128, 512], f32, tag="os", name="os")
                    nc.vector.scalar_tensor_tensor(
                        out=os_, in0=mp, scalar=bm2_sb[:, c2t:c2t + 1],
                        in1=xf[:, c2t, cs], op0=ADD, op1=ADD)
                    qeng = (nc.sync, nc.scalar, nc.gpsimd, nc.sync)[2 * c + c2t]
                    qeng.dma_start(
                        out=y.rearrange("(a p) t -> p a t", p=128)[:, c2t, cs],
                        in_=os_,
                    )

    nc.compile()
    return nc


def _host_inputs(image, w_kqv, b_kqv, w_mix, b_mix):
    s = np.float32(1.0 / np.sqrt(D))
    wk = w_kqv[:, :256]
    wq = w_kqv[:, 256:512] * s
    wv = w_kqv[:, 512:]
    bk = b_kqv[:256].astype(np.float32)
    bq = (b_kqv[256:512] * s).astype(np.float32)
    bv = b_kqv[512:].astype(np.float32)
    bjt = np.stack([bk[0:128], bk[128:256], bq[0:128], bq[128:256]])
    bm_eff = (np.asarray(b_mix, np.float32)
              + bv @ np.asarray(w_mix, np.float32))
    bm2 = bm_eff.reshape(2, 128)

    idx = np.arange(128)
    mask = (idx[:, None] > idx[None, :]).astype(np.float32)  # s > t
    tri = 1.0 - mask                                         # keep mask
    mbf = np.full((128, 512), EXP_B, np.float32)
    mbf[:, 0:128] = EXP_B - (MASK60 * EXP_A) * mask

    # selp[rs_row, gp, po_row]: rs row r = 4g+2h2+pair holds head 4g+2pair+h2
    selp = np.zeros((8, 4, 128), np.float32)
    # wmp[po_row, gp, out_chan]
    wmp = np.zeros((128, 4, 256), np.float32)
    wm = np.asarray(w_mix, np.float32)
    for g in range(2):
        for pair in range(2):
            gp = 2 * g + pair
            for h2 in range(2):
                r = 4 * g + 2 * h2 + pair
                selp[r, gp, 64 * h2:64 * h2 + 33] = 1.0
                in0 = 128 * g + 32 * (2 * pair + h2)
                wmp[64 * h2:64 * h2 + 32, gp, :] = wm[in0:in0 + 32, :]

    common = {
        "wk": np.ascontiguousarray(wk).astype(BF),
        "wq": np.ascontiguousarray(wq).astype(BF),
        "wv": np.ascontiguousarray(wv).astype(BF),
        "wmp": wmp.astype(BF),
        "bjt": np.ascontiguousarray(bjt),
        "bm2": np.ascontiguousarray(bm2),
        "tri": tri.astype(BF),
        "mbf": mbf,
        "selp": selp.astype(BF),
    }
    in_maps = []
    for i in range(N_CORES):
        x = np.ascontiguousarray(image[i].reshape(C, T)).astype(np.float32)
        in_maps.append({**common, "x_f": x, "x_bf": x.astype(BF)})
    return in_maps


def _run(inputs, trace=False):
    if "nc" not in _CACHE:
        _CACHE["nc"] = _build_nc()
    nc = _CACHE["nc"]
    in_maps = _host_inputs(
        np.asarray(inputs["image"], np.float32),
        np.asarray(inputs["w_kqv"], np.float32),
        np.asarray(inputs["b_kqv"], np.float32),
        np.asarray(inputs["w_mix"], np.float32),
        np.asarray(inputs["b_mix"], np.float32),
    )
    res = run_bass_kernel_spmd(nc, in_maps, list(range(N_CORES)), trace=trace)
    out = np.stack(
        [np.asarray(res.results[i]["y"]).reshape(C, 32, 32) for i in range(N_CORES)]
    ).astype(np.float32)
    return out, res


def kernel(**inputs):
    out, _ = _run(inputs, trace=False)
    return out
